# revision 17
# baseline (speedup 1.0000x reference)
"""Causal GQA attention (B=2, S=2048, 32 q-heads, 8 kv-heads, d=128) on 8 trn2 cores.

Sharding: tensor-parallel on the head axis. Core c gets kv-head c and its 4
query heads (cols [c*512,(c+1)*512) of q / [c*128,(c+1)*128) of k,v). Each core
runs a full causal flash-attention over its heads; outputs are concatenated.

Per-core algorithm (per batch b, per 128-row q-tile i, per head j):
  scores[q,k] = (Q K^T) computed as matmul(lhsT=Q^T_tile, rhs=K^T chunks) into
  PSUM; the causal mask for the diagonal 128x128 tile is added by an extra
  matmul (lhsT=I, rhs=upper-tri(-1e9)); softmax without max-subtraction
  (scores ~ N(0,1), exp is safe in fp32): one ACT pass computes
  P = exp(scale*scores) in bf16 with accum_out giving the row sums; P tiles are
  transposed on the PE (bf16, via identity) and fed as stationary operands to
  the PV matmuls accumulating O[q,d] in PSUM; the final PSUM->SBUF copy scales
  by 1/rowsum (per-partition scalar on ACT).
"""

import numpy as np

B = 2
S = 2048
D = 128
N_CORES = 8
HQ_PER_CORE = 4
DQ = HQ_PER_CORE * D  # 512
SCALING = D ** -0.5
NEG = -1e9
PTILE = 128


DEFAULT_BUFS = dict(
    inbuf=2, ktbuf=2, qtbuf=3, pbuf=4, ptbuf=4, obuf=3, stats=12,
    ps_s=2, ps_t=3, ps_o=1,
)

# scheme B (transposed scores): all PE matmuls use 512-wide moving operands
DEFAULT_BUFS_B = dict(
    inbuf=2, ktbuf=2, qtbuf=3, pbuf=3, ptbuf=0, obuf=3, stats=12, accbuf=2,
    otbuf=2, ps_s=2, ps_t=1, ps_o=2, ps_sum=1,
)


def build_program(
    seq_len=S, batch=B, n_heads=HQ_PER_CORE, *, num_devices=N_CORES, reps=1,
    bufs=None,
):
    """Build (and bacc-compile) the single-core Bass program run SPMD on all cores.

    reps>1 wraps the whole computation in a dynamic loop (for on-device timing).
    """
    from contextlib import ExitStack

    import concourse.bass as bass  # noqa: F401
    import concourse.mybir as mybir
    import concourse.tile as tile
    from concourse import bacc
    from concourse.masks import make_causal_mask, make_identity

    f32 = mybir.dt.float32
    bf16 = mybir.dt.bfloat16
    AF = mybir.ActivationFunctionType

    dq = n_heads * D
    nt = seq_len // PTILE  # number of 128-row tiles along sequence

    nc = bacc.Bacc(
        "TRN2", target_bir_lowering=False, debug=False, num_devices=num_devices
    )
    q = nc.dram_tensor("q", [batch, seq_len, dq], f32, kind="ExternalInput").ap()
    k = nc.dram_tensor("k", [batch, seq_len, D], f32, kind="ExternalInput").ap()
    v = nc.dram_tensor("v", [batch, seq_len, D], f32, kind="ExternalInput").ap()
    o = nc.dram_tensor("o", [batch, seq_len, dq], f32, kind="ExternalOutput").ap()

    def transpose_group(nc, out_ps, srcs, ident):
        """Transpose each [128,128] src into its 128-col slice of one PSUM bank."""
        n = len(srcs)
        for g, src in enumerate(srcs):
            nc.tensor.matmul(
                out_ps[:, g * 128 : (g + 1) * 128],
                src,
                ident,
                is_transpose=True,
                start=(g == 0),
                stop=(g == n - 1),
            )

    bf = dict(DEFAULT_BUFS)
    if bufs:
        bf.update(bufs)

    with tile.TileContext(nc) as tc, ExitStack() as ctx:
        const = ctx.enter_context(tc.tile_pool(name="const", bufs=1))
        inbuf = ctx.enter_context(tc.tile_pool(name="inbuf", bufs=bf["inbuf"]))
        ktbuf = ctx.enter_context(tc.tile_pool(name="ktbuf", bufs=bf["ktbuf"]))
        qtbuf = ctx.enter_context(tc.tile_pool(name="qtbuf", bufs=bf["qtbuf"]))
        pbuf = ctx.enter_context(tc.tile_pool(name="pbuf", bufs=bf["pbuf"]))
        ptbuf = ctx.enter_context(tc.tile_pool(name="ptbuf", bufs=bf["ptbuf"]))
        obuf = ctx.enter_context(tc.tile_pool(name="obuf", bufs=bf["obuf"]))
        stats = ctx.enter_context(tc.tile_pool(name="stats", bufs=bf["stats"]))
        ps_s = ctx.enter_context(
            tc.tile_pool(name="ps_s", bufs=bf["ps_s"], space="PSUM")
        )
        ps_t = ctx.enter_context(
            tc.tile_pool(name="ps_t", bufs=bf["ps_t"], space="PSUM")
        )
        ps_o = ctx.enter_context(
            tc.tile_pool(name="ps_o", bufs=bf["ps_o"], space="PSUM")
        )

        ident = const.tile([128, 128], bf16)
        make_identity(nc, ident)
        # mask[r, c] = 0 for c <= r, -1e9 for c > r (future keys masked)
        maskt = const.tile([128, 128], bf16)
        make_causal_mask(nc, maskt, mask_val=NEG)

        def body(_it=None):
            _attention_body(
                nc, tc, mybir, AF, q, k, v, o,
                inbuf, ktbuf, qtbuf, pbuf, ptbuf, obuf, stats,
                ps_s, ps_t, ps_o,
                ident, maskt, transpose_group,
                batch, seq_len, nt, dq, n_heads, f32, bf16,
            )

        if reps > 1:
            with tc.For_i(0, reps, 1) as _it:
                body(_it)
        else:
            body()

    nc.compile()
    return nc


def _attention_body(
    nc, tc, mybir, AF, q, k, v, o,
    inbuf, ktbuf, qtbuf, pbuf, ptbuf, obuf, stats,
    ps_s, ps_t, ps_o,
    ident, maskt, transpose_group,
    batch, seq_len, nt, dq, n_heads, f32, bf16,
):
    for b in range(batch):
        q_sb = inbuf.tile([128, nt, dq], bf16, tag="q_sb")
        nc.gpsimd.dma_start(out=q_sb, in_=q[b].rearrange("(t p) d -> p t d", p=128))
        k_sb = inbuf.tile([128, nt, D], bf16, tag="k_sb")
        nc.gpsimd.dma_start(out=k_sb, in_=k[b].rearrange("(t p) d -> p t d", p=128))
        v_sb = inbuf.tile([128, nt, D], bf16, tag="v_sb")
        nc.gpsimd.dma_start(out=v_sb, in_=v[b].rearrange("(t p) d -> p t d", p=128))

        # K^T [d, s] once per batch, transposes batched 8-per-PSUM-bank
        kt_sb = ktbuf.tile([128, seq_len], bf16)
        for t0 in range(0, nt, 8):
            gn = min(8, nt - t0)
            ktp = ps_t.tile([128, 1024], bf16, tag="tp")
            transpose_group(
                nc, ktp, [k_sb[:, t0 + g, :] for g in range(gn)], ident
            )
            nc.vector.tensor_copy(
                kt_sb[:, t0 * 128 : (t0 + gn) * 128], ktp[:, : gn * 128]
            )

        # --- software-pipelined (i, j) items: scores emitted LOOKAHEAD items
        # ahead of the PV phase so PE/ACT/DVE always have independent work ---
        state = {}  # (i, j) -> dict with p_sb, recip, o_sb, qt_sb, lk

        def emit_scores(i, j):
            lk = (i + 1) * 128
            if j == 0:
                qt_sb = qtbuf.tile([128, dq], bf16, tag="qt_sb")
                for j0 in range(0, n_heads, 8):
                    gn = min(8, n_heads - j0)
                    qtp = ps_t.tile([128, 1024], bf16, tag="tp")
                    transpose_group(
                        nc,
                        qtp,
                        [
                            q_sb[:, i, (j0 + g) * 128 : (j0 + g + 1) * 128]
                            for g in range(gn)
                        ],
                        ident,
                    )
                    nc.vector.tensor_copy(
                        qt_sb[:, j0 * 128 : (j0 + gn) * 128], qtp[:, : gn * 128]
                    )
                o_sb = obuf.tile([128, dq], f32, tag="o_sb")
                state[(i, "row")] = (qt_sb, o_sb)
            qt_sb, o_sb = state[(i, "row")]

            nch = (lk + 1023) // 1024  # 1024-wide score tiles (2 PSUM banks)
            p_sb = pbuf.tile([128, seq_len], bf16, tag="p_sb")
            csum = stats.tile([128, 2], f32, tag="csum")
            for c in range(nch):
                w = min(1024, lk - c * 1024)
                s_ps = ps_s.tile([128, 1024], f32, tag="s")
                nsub = (w + 511) // 512
                for u in range(nsub):
                    uw = min(512, w - u * 512)
                    kbase = c * 1024 + u * 512
                    has_diag = i * 128 >= kbase and i * 128 < kbase + uw
                    nc.tensor.matmul(
                        s_ps[:, u * 512 : u * 512 + uw],
                        qt_sb[:, j * 128 : (j + 1) * 128],
                        kt_sb[:, kbase : kbase + uw],
                        start=True,
                        stop=not has_diag,
                    )
                    if has_diag:
                        off = i * 128 - kbase
                        nc.tensor.matmul(
                            s_ps[:, u * 512 + off : u * 512 + off + 128],
                            ident,
                            maskt,
                            start=False,
                            stop=True,
                        )
                nc.scalar.activation(
                    p_sb[:, c * 1024 : c * 1024 + w],
                    s_ps[:, :w],
                    AF.Exp,
                    bias=0.0,
                    scale=SCALING,
                    accum_out=csum[:, c : c + 1],
                )
            sums = stats.tile([128, 1], f32, tag="sums")
            nc.vector.reduce_sum(sums, csum[:, :nch], axis=mybir.AxisListType.X)
            recip = stats.tile([128, 1], f32, tag="recip")
            nc.vector.reciprocal(recip, sums)
            state[(i, j)] = (p_sb, recip)

        def emit_pv(i, j):
            p_sb, recip = state.pop((i, j))
            qt_sb, o_sb = state[(i, "row")]
            o_ps = ps_o.tile([128, 128], f32, tag="o")
            for t0 in range(0, i + 1, 8):
                gn = min(8, i + 1 - t0)
                ptp = ps_t.tile([128, 1024], bf16, tag="tp")
                transpose_group(
                    nc,
                    ptp,
                    [
                        p_sb[:, (t0 + g) * 128 : (t0 + g + 1) * 128]
                        for g in range(gn)
                    ],
                    ident,
                )
                pt_sb = ptbuf.tile([128, 1024], bf16, tag="pt_sb")
                nc.vector.tensor_copy(pt_sb[:, : gn * 128], ptp[:, : gn * 128])
                for g in range(gn):
                    t = t0 + g
                    nc.tensor.matmul(
                        o_ps,
                        pt_sb[:, g * 128 : (g + 1) * 128],
                        v_sb[:, t, :],
                        start=(t == 0),
                        stop=(t == i),
                    )
            nc.vector.tensor_scalar_mul(
                o_sb[:, j * 128 : (j + 1) * 128], o_ps, recip
            )
            if j == n_heads - 1:
                del state[(i, "row")]
                nc.sync.dma_start(out=o[b, i * 128 : (i + 1) * 128, :], in_=o_sb)

        items = [(i, j) for i in range(nt) for j in range(n_heads)]
        LOOKAHEAD = 2
        for n in range(min(LOOKAHEAD, len(items))):
            emit_scores(*items[n])
        for n, it in enumerate(items):
            if n + LOOKAHEAD < len(items):
                emit_scores(*items[n + LOOKAHEAD])
            emit_pv(*it)


def build_program_b(
    seq_len=S, batch=B, n_heads=HQ_PER_CORE, *, num_devices=N_CORES, reps=1,
    bufs=None,
):
    """Scheme B: scores computed transposed (S^T[k, q]) with 512-wide moving
    operands spanning all heads, so every stationary load is amortized 4x and
    the P-transposes of scheme A disappear. Row sums come from a DVE add-tree
    over the P^T tiles plus tiny per-head ones-matmuls; O^T is transposed back
    on the PE at the end."""
    from contextlib import ExitStack

    import concourse.mybir as mybir
    import concourse.tile as tile
    from concourse import bacc
    from concourse.masks import make_causal_mask, make_identity

    f32 = mybir.dt.float32
    bf16 = mybir.dt.bfloat16
    AF = mybir.ActivationFunctionType

    dq = n_heads * D
    nt = seq_len // PTILE

    bf = dict(
        inbuf=2, ktbuf=2, qtbuf=3, pbuf=4, pairbuf=6, accbuf=2, otbuf=2,
        obuf=3, stats=8, ps_s=2, ps_ot=2, ps_tp=1, ps_sum=1,
    )
    if bufs:
        bf.update(bufs)

    nc = bacc.Bacc(
        "TRN2", target_bir_lowering=False, debug=False, num_devices=num_devices
    )
    q = nc.dram_tensor("q", [batch, seq_len, dq], f32, kind="ExternalInput").ap()
    k = nc.dram_tensor("k", [batch, seq_len, D], f32, kind="ExternalInput").ap()
    v = nc.dram_tensor("v", [batch, seq_len, D], f32, kind="ExternalInput").ap()
    o = nc.dram_tensor("o", [batch, seq_len, dq], f32, kind="ExternalOutput").ap()

    def transpose_group(out_ps, srcs, identity):
        n = len(srcs)
        for g, src in enumerate(srcs):
            nc.tensor.matmul(
                out_ps[:, g * 128 : (g + 1) * 128],
                src,
                identity,
                is_transpose=True,
                start=(g == 0),
                stop=(g == n - 1),
            )

    with tile.TileContext(nc) as tc, ExitStack() as ctx:
        const = ctx.enter_context(tc.tile_pool(name="const", bufs=1))
        inbuf = ctx.enter_context(tc.tile_pool(name="inbuf", bufs=bf["inbuf"]))
        ktbuf = ctx.enter_context(tc.tile_pool(name="ktbuf", bufs=bf["ktbuf"]))
        qtbuf = ctx.enter_context(tc.tile_pool(name="qtbuf", bufs=bf["qtbuf"]))
        pbuf = ctx.enter_context(tc.tile_pool(name="pbuf", bufs=bf["pbuf"]))
        pairbuf = ctx.enter_context(tc.tile_pool(name="pairbuf", bufs=bf["pairbuf"]))
        accbuf = ctx.enter_context(tc.tile_pool(name="accbuf", bufs=bf["accbuf"]))
        otbuf = ctx.enter_context(tc.tile_pool(name="otbuf", bufs=bf["otbuf"]))
        obuf = ctx.enter_context(tc.tile_pool(name="obuf", bufs=bf["obuf"]))
        stats = ctx.enter_context(tc.tile_pool(name="stats", bufs=bf["stats"]))
        ps_s = ctx.enter_context(tc.tile_pool(name="ps_s", bufs=bf["ps_s"], space="PSUM"))
        ps_ot = ctx.enter_context(tc.tile_pool(name="ps_ot", bufs=bf["ps_ot"], space="PSUM"))
        ps_tp = ctx.enter_context(tc.tile_pool(name="ps_tp", bufs=bf["ps_tp"], space="PSUM"))
        ps_sum = ctx.enter_context(tc.tile_pool(name="ps_sum", bufs=bf["ps_sum"], space="PSUM"))

        ident = const.tile([128, 128], bf16)
        make_identity(nc, ident)
        ident_f32 = const.tile([128, 128], f32)
        make_identity(nc, ident_f32)
        # 4 identity blocks side by side: mask matmul covers all heads at once
        ident_rep = const.tile([128, n_heads * 128], bf16)
        for j in range(n_heads):
            make_identity(nc, ident_rep[:, j * 128 : (j + 1) * 128])
        maskt = const.tile([128, 128], bf16)
        make_causal_mask(nc, maskt, mask_val=NEG)
        ones_f32 = const.tile([128, 1], f32)
        nc.vector.memset(ones_f32, 1.0)

        def body(_it=None):
            # load every batch up front; the two batches' work is interleaved
            # pair-by-pair so ACT/PE/DVE always have an independent stream
            q_sbs, v_sbs, kt_sbs = [], [], []
            for b in range(batch):
                q_sb = inbuf.tile([128, nt, dq], bf16, tag="q_sb")
                nc.gpsimd.dma_start(
                    out=q_sb, in_=q[b].rearrange("(t p) d -> p t d", p=128)
                )
                k_sb = inbuf.tile([128, nt, D], bf16, tag="k_sb")
                nc.gpsimd.dma_start(
                    out=k_sb, in_=k[b].rearrange("(t p) d -> p t d", p=128)
                )
                v_sb = inbuf.tile([128, nt, D], bf16, tag="v_sb")
                nc.gpsimd.dma_start(
                    out=v_sb, in_=v[b].rearrange("(t p) d -> p t d", p=128)
                )
                kt_sb = ktbuf.tile([128, seq_len], bf16, tag="kt_sb")
                for t0 in range(0, nt, 8):
                    gn = min(8, nt - t0)
                    ktp = ps_tp.tile([128, 1024], bf16, tag="tp")
                    transpose_group(
                        ktp, [k_sb[:, t0 + g, :] for g in range(gn)], ident
                    )
                    nc.vector.tensor_copy(
                        kt_sb[:, t0 * 128 : (t0 + gn) * 128], ktp[:, : gn * 128]
                    )
                q_sbs.append(q_sb)
                v_sbs.append(v_sb)
                kt_sbs.append(kt_sb)

            class Item:
                """Per-(b, i) stream state with score/PV pair emitters."""

                def __init__(self, b, i):
                    self.b, self.i = b, i
                    self.npair = (i + 2) // 2
                    self.pts = {}
                    self.pending = []
                    qt_sb = qtbuf.tile([128, dq], bf16, tag="qt_sb")
                    qtp = ps_tp.tile([128, 1024], bf16, tag="tp")
                    transpose_group(
                        qtp,
                        [
                            q_sbs[b][:, i, jj * 128 : (jj + 1) * 128]
                            for jj in range(n_heads)
                        ],
                        ident,
                    )
                    nc.vector.tensor_copy(qt_sb, qtp[:, :dq])
                    self.qt_sb = qt_sb
                    self.acc = accbuf.tile([128, dq], f32, tag="acc")
                    self.ot_ps = ps_ot.tile([128, dq], f32, tag="ot")

                def fold_partial(self, part):
                    if self.pending:
                        prev, depth = self.pending.pop()
                        if depth < 2:
                            comb = pairbuf.tile([128, dq], bf16, tag="pair")
                            nc.gpsimd.tensor_add(comb, prev, part)
                            self.pending.append((comb, depth + 1))
                        else:
                            nc.vector.tensor_add(self.acc, self.acc, prev)
                            self.pending.append((part, 1))
                    else:
                        self.pending.append((part, 1))

                def emit_scores_pair(self, p):
                    i = self.i
                    ts_ = [t for t in (2 * p, 2 * p + 1) if t <= i]
                    w = len(ts_) * dq
                    st = ps_s.tile([128, 2 * dq], f32, tag="st")
                    for s, t in enumerate(ts_):
                        nc.tensor.matmul(
                            st[:, s * dq : s * dq + dq],
                            kt_sbs[self.b][:, t * 128 : (t + 1) * 128],
                            self.qt_sb,
                            start=True,
                            stop=(t != i),
                        )
                        if t == i:
                            nc.tensor.matmul(
                                st[:, s * dq : s * dq + dq],
                                maskt,
                                ident_rep,
                                start=False,
                                stop=True,
                            )
                    pt = pbuf.tile([128, 2 * dq], bf16, tag="pt")
                    nc.scalar.activation(
                        pt[:, :w], st[:, :w], AF.Exp, bias=0.0, scale=SCALING
                    )
                    self.pts[p] = pt
                    acc = self.acc
                    if len(ts_) == 2:
                        if p == 0:
                            nc.vector.tensor_add(
                                acc, pt[:, :dq], pt[:, dq : 2 * dq]
                            )
                        else:
                            tmp = pairbuf.tile([128, dq], bf16, tag="pair")
                            nc.gpsimd.tensor_add(
                                tmp, pt[:, :dq], pt[:, dq : 2 * dq]
                            )
                            self.fold_partial(tmp)
                    else:
                        if p == 0:
                            nc.vector.tensor_copy(acc, pt[:, :dq])
                        else:
                            self.fold_partial(pt[:, :dq])

                def emit_pv_pair(self, p):
                    i = self.i
                    pt = self.pts.pop(p)
                    for s, t in enumerate(
                        tt for tt in (2 * p, 2 * p + 1) if tt <= i
                    ):
                        nc.tensor.matmul(
                            self.ot_ps,
                            v_sbs[self.b][:, t, :],
                            pt[:, s * dq : s * dq + dq],
                            start=(t == 0),
                            stop=(t == i),
                        )

                def finish_acc(self):
                    for part, _ in self.pending:
                        nc.vector.tensor_add(self.acc, self.acc, part)
                    self.pending = []

                def emit_tail(self):
                    i = self.i
                    o_sb = obuf.tile([128, dq], f32, tag="o_sb")
                    sums_ps = ps_sum.tile([128, n_heads], f32, tag="sums")
                    for j in range(n_heads):
                        nc.tensor.matmul(
                            sums_ps[:, j : j + 1],
                            self.acc[:, j * 128 : (j + 1) * 128],
                            ones_f32,
                            start=(j == 0),
                            stop=(j == n_heads - 1),
                        )
                    recip = stats.tile([128, n_heads], f32, tag="recip")
                    nc.vector.reciprocal(recip, sums_ps)

                    ot_sb = otbuf.tile([128, dq], f32, tag="ot_sb")
                    nc.vector.tensor_copy(ot_sb, self.ot_ps)
                    otr = ps_tp.tile([128, dq], f32, tag="tp")
                    transpose_group(
                        otr,
                        [
                            ot_sb[:, j * 128 : (j + 1) * 128]
                            for j in range(n_heads)
                        ],
                        ident_f32,
                    )
                    for j in range(n_heads):
                        nc.vector.tensor_scalar_mul(
                            o_sb[:, j * 128 : (j + 1) * 128],
                            otr[:, j * 128 : (j + 1) * 128],
                            recip[:, j : j + 1],
                        )
                    nc.sync.dma_start(
                        out=o[self.b, i * 128 : (i + 1) * 128, :], in_=o_sb
                    )

            pending_tail = None
            for b in range(batch):
                for i in range(nt):
                    it = Item(b, i)
                    it.emit_scores_pair(0)
                    for p in range(it.npair):
                        if p + 1 < it.npair:
                            it.emit_scores_pair(p + 1)
                        it.emit_pv_pair(p)
                    it.finish_acc()
                    if pending_tail is not None:
                        pending_tail.emit_tail()
                    pending_tail = it
            pending_tail.emit_tail()

        if reps > 1:
            with tc.For_i(0, reps, 1) as _it:
                body(_it)
        else:
            body()

    nc.compile()
    return nc


_PROGRAM = None
SCHEME = "A"


def _get_program():
    global _PROGRAM
    if _PROGRAM is None:
        if SCHEME == "B":
            _PROGRAM = build_program_b()
        else:
            _PROGRAM = build_program()
    return _PROGRAM


def kernel(query, key, value):
    from concourse.bass_utils import run_bass_kernel_spmd

    query = np.asarray(query, dtype=np.float32)
    key = np.asarray(key, dtype=np.float32)
    value = np.asarray(value, dtype=np.float32)
    assert query.shape == (B, S, N_CORES * DQ), query.shape
    assert key.shape == (B, S, N_CORES * D), key.shape
    assert value.shape == (B, S, N_CORES * D), value.shape

    nc = _get_program()
    in_maps = [
        {
            "q": np.ascontiguousarray(query[:, :, c * DQ : (c + 1) * DQ]),
            "k": np.ascontiguousarray(key[:, :, c * D : (c + 1) * D]),
            "v": np.ascontiguousarray(value[:, :, c * D : (c + 1) * D]),
        }
        for c in range(N_CORES)
    ]
    res = run_bass_kernel_spmd(nc, in_maps, core_ids=list(range(N_CORES)))
    out = np.empty((B, S, N_CORES * DQ), dtype=np.float32)
    for c in range(N_CORES):
        out[:, :, c * DQ : (c + 1) * DQ] = res.results[c]["o"]
    return out


# revision 33
# speedup vs baseline: 2.2248x; 2.2248x over previous
"""Causal GQA attention (B=2, S=2048, 32 q-heads, 8 kv-heads, d=128) on 8 trn2 cores.

Sharding: tensor-parallel on the head axis. Core c gets kv-head c and its 4
query heads (cols [c*512,(c+1)*512) of q / [c*128,(c+1)*128) of k,v). Each core
runs a full causal flash-attention over its heads; outputs are concatenated.

Per-core algorithm (per batch b, per 128-row q-tile i, per head j):
  scores[q,k] = (Q K^T) computed as matmul(lhsT=Q^T_tile, rhs=K^T chunks) into
  PSUM; the causal mask for the diagonal 128x128 tile is added by an extra
  matmul (lhsT=I, rhs=upper-tri(-1e9)); softmax without max-subtraction
  (scores ~ N(0,1), exp is safe in fp32): one ACT pass computes
  P = exp(scale*scores) in bf16 with accum_out giving the row sums; P tiles are
  transposed on the PE (bf16, via identity) and fed as stationary operands to
  the PV matmuls accumulating O[q,d] in PSUM; the final PSUM->SBUF copy scales
  by 1/rowsum (per-partition scalar on ACT).
"""

import numpy as np

B = 2
S = 2048
D = 128
N_CORES = 8
HQ_PER_CORE = 4
DQ = HQ_PER_CORE * D  # 512
SCALING = D ** -0.5
NEG = -1e9
PTILE = 128


DEFAULT_BUFS = dict(
    inbuf=2, ktbuf=2, qtbuf=3, pbuf=6, ptbuf=4, obuf=3, stats=16,
    ps_s=2, ps_t=3, ps_o=1,
)

# scheme B (transposed scores): all PE matmuls use 512-wide moving operands
DEFAULT_BUFS_B = dict(
    inbuf=2, ktbuf=2, qtbuf=3, pbuf=3, ptbuf=0, obuf=3, stats=12, accbuf=2,
    otbuf=2, ps_s=2, ps_t=1, ps_o=2, ps_sum=1,
)


def build_program(
    seq_len=S, batch=B, n_heads=HQ_PER_CORE, *, num_devices=N_CORES, reps=1,
    bufs=None, lookahead=4, schunk=1024,
):
    """Build (and bacc-compile) the single-core Bass program run SPMD on all cores.

    reps>1 wraps the whole computation in a dynamic loop (for on-device timing).
    """
    from contextlib import ExitStack

    import concourse.bass as bass  # noqa: F401
    import concourse.mybir as mybir
    import concourse.tile as tile
    from concourse import bacc
    from concourse.masks import make_causal_mask, make_identity

    f32 = mybir.dt.float32
    bf16 = mybir.dt.bfloat16
    AF = mybir.ActivationFunctionType

    dq = n_heads * D
    nt = seq_len // PTILE  # number of 128-row tiles along sequence

    nc = bacc.Bacc(
        "TRN2", target_bir_lowering=False, debug=False, num_devices=num_devices
    )
    q = nc.dram_tensor("q", [batch, seq_len, dq], f32, kind="ExternalInput").ap()
    k = nc.dram_tensor("k", [batch, seq_len, D], f32, kind="ExternalInput").ap()
    v = nc.dram_tensor("v", [batch, seq_len, D], f32, kind="ExternalInput").ap()
    o = nc.dram_tensor("o", [batch, seq_len, dq], f32, kind="ExternalOutput").ap()

    def transpose_group(nc, out_ps, srcs, ident):
        """Transpose each [128,128] src into its 128-col slice of one PSUM bank."""
        n = len(srcs)
        for g, src in enumerate(srcs):
            nc.tensor.matmul(
                out_ps[:, g * 128 : (g + 1) * 128],
                src,
                ident,
                is_transpose=True,
                start=(g == 0),
                stop=(g == n - 1),
            )

    bf = dict(DEFAULT_BUFS)
    if bufs:
        bf.update(bufs)

    with tile.TileContext(nc) as tc, ExitStack() as ctx:
        const = ctx.enter_context(tc.tile_pool(name="const", bufs=1))
        inbuf = ctx.enter_context(tc.tile_pool(name="inbuf", bufs=bf["inbuf"]))
        ktbuf = ctx.enter_context(tc.tile_pool(name="ktbuf", bufs=bf["ktbuf"]))
        qtbuf = ctx.enter_context(tc.tile_pool(name="qtbuf", bufs=bf["qtbuf"]))
        pbuf = ctx.enter_context(tc.tile_pool(name="pbuf", bufs=bf["pbuf"]))
        ptbuf = ctx.enter_context(tc.tile_pool(name="ptbuf", bufs=bf["ptbuf"]))
        obuf = ctx.enter_context(tc.tile_pool(name="obuf", bufs=bf["obuf"]))
        stats = ctx.enter_context(tc.tile_pool(name="stats", bufs=bf["stats"]))
        ps_s = ctx.enter_context(
            tc.tile_pool(name="ps_s", bufs=bf["ps_s"], space="PSUM")
        )
        ps_t = ctx.enter_context(
            tc.tile_pool(name="ps_t", bufs=bf["ps_t"], space="PSUM")
        )
        ps_o = ctx.enter_context(
            tc.tile_pool(name="ps_o", bufs=bf["ps_o"], space="PSUM")
        )

        ident = const.tile([128, 128], bf16)
        make_identity(nc, ident)
        # mask[r, c] = 0 for c <= r, -1e9 for c > r (future keys masked)
        maskt = const.tile([128, 128], bf16)
        make_causal_mask(nc, maskt, mask_val=NEG)

        def body(_it=None):
            _attention_body(
                nc, tc, mybir, AF, q, k, v, o,
                inbuf, ktbuf, qtbuf, pbuf, ptbuf, obuf, stats,
                ps_s, ps_t, ps_o,
                ident, maskt, transpose_group,
                batch, seq_len, nt, dq, n_heads, f32, bf16,
                lookahead=lookahead, schunk=schunk,
            )

        if reps > 1:
            hint = (
                mybir.EngineType.PE,
                mybir.EngineType.Activation,
                mybir.EngineType.DVE,
                mybir.EngineType.Pool,
                mybir.EngineType.SP,
            )
            with tc.For_i(0, reps, 1, hint_engines=hint) as _it:
                body(_it)
        else:
            body()

    nc.compile()
    return nc


def _attention_body(
    nc, tc, mybir, AF, q, k, v, o,
    inbuf, ktbuf, qtbuf, pbuf, ptbuf, obuf, stats,
    ps_s, ps_t, ps_o,
    ident, maskt, transpose_group,
    batch, seq_len, nt, dq, n_heads, f32, bf16,
    lookahead=2, schunk=1024,
):
    q_sbs, v_sbs, kt_sbs = [], [], []
    half = nt // 2 if nt > 1 else nt
    for b in range(batch):
        # split input DMAs in half so transposes/QK start before the full load
        q_sb = inbuf.tile([128, nt, dq], bf16, tag="q_sb")
        qv = q[b].rearrange("(t p) d -> p t d", p=128)
        nc.gpsimd.dma_start(out=q_sb[:, :half], in_=qv[:, :half])
        if half < nt:
            nc.gpsimd.dma_start(out=q_sb[:, half:], in_=qv[:, half:])
        k_sb = inbuf.tile([128, nt, D], bf16, tag="k_sb")
        kv_ = k[b].rearrange("(t p) d -> p t d", p=128)
        nc.gpsimd.dma_start(out=k_sb[:, :half], in_=kv_[:, :half])
        if half < nt:
            nc.gpsimd.dma_start(out=k_sb[:, half:], in_=kv_[:, half:])
        v_sb = inbuf.tile([128, nt, D], bf16, tag="v_sb")
        vv = v[b].rearrange("(t p) d -> p t d", p=128)
        nc.gpsimd.dma_start(out=v_sb[:, :half], in_=vv[:, :half])
        if half < nt:
            nc.gpsimd.dma_start(out=v_sb[:, half:], in_=vv[:, half:])

        # K^T [d, s] once per batch, transposes batched 8-per-PSUM-bank
        kt_sb = ktbuf.tile([128, seq_len], bf16, tag="kt_sb")
        for t0 in range(0, nt, 8):
            gn = min(8, nt - t0)
            ktp = ps_t.tile([128, 1024], bf16, tag="tp")
            transpose_group(
                nc, ktp, [k_sb[:, t0 + g, :] for g in range(gn)], ident
            )
            nc.vector.tensor_copy(
                kt_sb[:, t0 * 128 : (t0 + gn) * 128], ktp[:, : gn * 128]
            )
        q_sbs.append(q_sb)
        v_sbs.append(v_sb)
        kt_sbs.append(kt_sb)

    # --- software-pipelined (b, i, j) items: scores emitted `lookahead` items
    # ahead of the PV phase so PE/ACT/DVE always have independent work ---
    state = {}
    nsb = schunk // 512  # 512-col sub-chunks per score tile

    def emit_qt(b, i, j):
        # row-level Q^T transposes, emitted with PE work in between so the
        # DVE qt copy lands before the row's first QK chunk needs it
        if j != 0:
            return
        qt_sb = qtbuf.tile([128, dq], bf16, tag="qt_sb")
        for j0 in range(0, n_heads, 8):
            gn = min(8, n_heads - j0)
            qtp = ps_t.tile([128, 1024], bf16, tag="tp")
            transpose_group(
                nc,
                qtp,
                [
                    q_sbs[b][:, i, (j0 + g) * 128 : (j0 + g + 1) * 128]
                    for g in range(gn)
                ],
                ident,
            )
            nc.vector.tensor_copy(
                qt_sb[:, j0 * 128 : (j0 + gn) * 128], qtp[:, : gn * 128]
            )
        o_sb = obuf.tile([128, dq], f32, tag="o_sb")
        state[(b, i, "row")] = (qt_sb, o_sb)

    def emit_scores(b, i, j):
        lk = (i + 1) * 128
        kt_sb = kt_sbs[b]
        qt_sb, o_sb = state[(b, i, "row")]

        nch = (lk + schunk - 1) // schunk
        p_sb = pbuf.tile([128, seq_len], bf16, tag="p_sb")
        csum = stats.tile([128, 4], f32, tag="csum")
        for c in range(nch):
            w = min(schunk, lk - c * schunk)
            s_ps = ps_s.tile([128, schunk], f32, tag="s")
            nsub = (w + 511) // 512
            for u in range(nsub):
                uw = min(512, w - u * 512)
                kbase = c * schunk + u * 512
                has_diag = i * 128 >= kbase and i * 128 < kbase + uw
                nc.tensor.matmul(
                    s_ps[:, u * 512 : u * 512 + uw],
                    qt_sb[:, j * 128 : (j + 1) * 128],
                    kt_sb[:, kbase : kbase + uw],
                    start=True,
                    stop=not has_diag,
                )
                if has_diag:
                    off = i * 128 - kbase
                    nc.tensor.matmul(
                        s_ps[:, u * 512 + off : u * 512 + off + 128],
                        ident,
                        maskt,
                        start=False,
                        stop=True,
                    )
            nc.scalar.activation(
                p_sb[:, c * schunk : c * schunk + w],
                s_ps[:, :w],
                AF.Exp,
                bias=0.0,
                scale=SCALING,
                accum_out=csum[:, c : c + 1],
            )
        sums = stats.tile([128, 1], f32, tag="sums")
        nc.vector.reduce_sum(sums, csum[:, :nch], axis=mybir.AxisListType.X)
        recip = stats.tile([128, 1], f32, tag="recip")
        nc.vector.reciprocal(recip, sums)
        state[(b, i, j)] = (p_sb, recip)

    def emit_pv(b, i, j, skew=2):
        p_sb, recip = state.pop((b, i, j))
        qt_sb, o_sb = state[(b, i, "row")]
        v_sb = v_sbs[b]
        o_ps = ps_o.tile([128, 128], f32, tag="o")
        groups = [(t0, min(8, i + 1 - t0)) for t0 in range(0, i + 1, 8)]
        pt_sbs = {}

        def emit_tcopy(gi):
            t0, gn = groups[gi]
            ptp = ps_t.tile([128, 1024], bf16, tag="tp")
            transpose_group(
                nc,
                ptp,
                [
                    p_sb[:, (t0 + g) * 128 : (t0 + g + 1) * 128]
                    for g in range(gn)
                ],
                ident,
            )
            pt_sb = ptbuf.tile([128, 1024], bf16, tag="pt_sb")
            nc.vector.tensor_copy(pt_sb[:, : gn * 128], ptp[:, : gn * 128])
            pt_sbs[gi] = pt_sb

        sk = min(skew, len(groups))
        for gi in range(sk):
            emit_tcopy(gi)
        for gi, (t0, gn) in enumerate(groups):
            if gi + sk < len(groups):
                emit_tcopy(gi + sk)
            pt_sb = pt_sbs.pop(gi)
            for g in range(gn):
                t = t0 + g
                nc.tensor.matmul(
                    o_ps,
                    pt_sb[:, g * 128 : (g + 1) * 128],
                    v_sb[:, t, :],
                    start=(t == 0),
                    stop=(t == i),
                )
        nc.vector.tensor_scalar_mul(
            o_sb[:, j * 128 : (j + 1) * 128], o_ps, recip
        )
        if j == n_heads - 1:
            del state[(b, i, "row")]
            nc.sync.dma_start(out=o[b, i * 128 : (i + 1) * 128, :], in_=o_sb)

    items = [
        (b, i, j) for b in range(batch) for i in range(nt) for j in range(n_heads)
    ]
    la = min(lookahead, len(items))
    for n in range(la):
        emit_qt(*items[n])
        emit_scores(*items[n])
    for n, it in enumerate(items):
        if n + la < len(items):
            emit_qt(*items[n + la])
            emit_scores(*items[n + la])
        emit_pv(*it)


def build_program_b(
    seq_len=S, batch=B, n_heads=HQ_PER_CORE, *, num_devices=N_CORES, reps=1,
    bufs=None,
):
    """Scheme B: scores computed transposed (S^T[k, q]) with 512-wide moving
    operands spanning all heads, so every stationary load is amortized 4x and
    the P-transposes of scheme A disappear. Row sums come from a DVE add-tree
    over the P^T tiles plus tiny per-head ones-matmuls; O^T is transposed back
    on the PE at the end."""
    from contextlib import ExitStack

    import concourse.mybir as mybir
    import concourse.tile as tile
    from concourse import bacc
    from concourse.masks import make_causal_mask, make_identity

    f32 = mybir.dt.float32
    bf16 = mybir.dt.bfloat16
    AF = mybir.ActivationFunctionType

    dq = n_heads * D
    nt = seq_len // PTILE

    bf = dict(
        inbuf=2, ktbuf=2, qtbuf=3, pbuf=4, pairbuf=6, accbuf=2, otbuf=2,
        obuf=3, stats=8, ps_s=2, ps_ot=2, ps_tp=1, ps_sum=1,
    )
    if bufs:
        bf.update(bufs)

    nc = bacc.Bacc(
        "TRN2", target_bir_lowering=False, debug=False, num_devices=num_devices
    )
    q = nc.dram_tensor("q", [batch, seq_len, dq], f32, kind="ExternalInput").ap()
    k = nc.dram_tensor("k", [batch, seq_len, D], f32, kind="ExternalInput").ap()
    v = nc.dram_tensor("v", [batch, seq_len, D], f32, kind="ExternalInput").ap()
    o = nc.dram_tensor("o", [batch, seq_len, dq], f32, kind="ExternalOutput").ap()

    def transpose_group(out_ps, srcs, identity):
        n = len(srcs)
        for g, src in enumerate(srcs):
            nc.tensor.matmul(
                out_ps[:, g * 128 : (g + 1) * 128],
                src,
                identity,
                is_transpose=True,
                start=(g == 0),
                stop=(g == n - 1),
            )

    with tile.TileContext(nc) as tc, ExitStack() as ctx:
        const = ctx.enter_context(tc.tile_pool(name="const", bufs=1))
        inbuf = ctx.enter_context(tc.tile_pool(name="inbuf", bufs=bf["inbuf"]))
        ktbuf = ctx.enter_context(tc.tile_pool(name="ktbuf", bufs=bf["ktbuf"]))
        qtbuf = ctx.enter_context(tc.tile_pool(name="qtbuf", bufs=bf["qtbuf"]))
        pbuf = ctx.enter_context(tc.tile_pool(name="pbuf", bufs=bf["pbuf"]))
        pairbuf = ctx.enter_context(tc.tile_pool(name="pairbuf", bufs=bf["pairbuf"]))
        accbuf = ctx.enter_context(tc.tile_pool(name="accbuf", bufs=bf["accbuf"]))
        otbuf = ctx.enter_context(tc.tile_pool(name="otbuf", bufs=bf["otbuf"]))
        obuf = ctx.enter_context(tc.tile_pool(name="obuf", bufs=bf["obuf"]))
        stats = ctx.enter_context(tc.tile_pool(name="stats", bufs=bf["stats"]))
        ps_s = ctx.enter_context(tc.tile_pool(name="ps_s", bufs=bf["ps_s"], space="PSUM"))
        ps_ot = ctx.enter_context(tc.tile_pool(name="ps_ot", bufs=bf["ps_ot"], space="PSUM"))
        ps_tp = ctx.enter_context(tc.tile_pool(name="ps_tp", bufs=bf["ps_tp"], space="PSUM"))
        ps_sum = ctx.enter_context(tc.tile_pool(name="ps_sum", bufs=bf["ps_sum"], space="PSUM"))

        ident = const.tile([128, 128], bf16)
        make_identity(nc, ident)
        ident_f32 = const.tile([128, 128], f32)
        make_identity(nc, ident_f32)
        # 4 identity blocks side by side: mask matmul covers all heads at once
        ident_rep = const.tile([128, n_heads * 128], bf16)
        for j in range(n_heads):
            make_identity(nc, ident_rep[:, j * 128 : (j + 1) * 128])
        maskt = const.tile([128, 128], bf16)
        make_causal_mask(nc, maskt, mask_val=NEG)
        ones_f32 = const.tile([128, 1], f32)
        nc.vector.memset(ones_f32, 1.0)

        def body(_it=None):
            # load every batch up front; the two batches' work is interleaved
            # pair-by-pair so ACT/PE/DVE always have an independent stream
            q_sbs, v_sbs, kt_sbs = [], [], []
            for b in range(batch):
                q_sb = inbuf.tile([128, nt, dq], bf16, tag="q_sb")
                nc.gpsimd.dma_start(
                    out=q_sb, in_=q[b].rearrange("(t p) d -> p t d", p=128)
                )
                k_sb = inbuf.tile([128, nt, D], bf16, tag="k_sb")
                nc.gpsimd.dma_start(
                    out=k_sb, in_=k[b].rearrange("(t p) d -> p t d", p=128)
                )
                v_sb = inbuf.tile([128, nt, D], bf16, tag="v_sb")
                nc.gpsimd.dma_start(
                    out=v_sb, in_=v[b].rearrange("(t p) d -> p t d", p=128)
                )
                kt_sb = ktbuf.tile([128, seq_len], bf16, tag="kt_sb")
                for t0 in range(0, nt, 8):
                    gn = min(8, nt - t0)
                    ktp = ps_tp.tile([128, 1024], bf16, tag="tp")
                    transpose_group(
                        ktp, [k_sb[:, t0 + g, :] for g in range(gn)], ident
                    )
                    nc.vector.tensor_copy(
                        kt_sb[:, t0 * 128 : (t0 + gn) * 128], ktp[:, : gn * 128]
                    )
                q_sbs.append(q_sb)
                v_sbs.append(v_sb)
                kt_sbs.append(kt_sb)

            class Item:
                """Per-(b, i) stream state with score/PV pair emitters."""

                def __init__(self, b, i):
                    self.b, self.i = b, i
                    self.npair = (i + 2) // 2
                    self.pts = {}
                    self.pending = []
                    qt_sb = qtbuf.tile([128, dq], bf16, tag="qt_sb")
                    qtp = ps_tp.tile([128, 1024], bf16, tag="tp")
                    transpose_group(
                        qtp,
                        [
                            q_sbs[b][:, i, jj * 128 : (jj + 1) * 128]
                            for jj in range(n_heads)
                        ],
                        ident,
                    )
                    nc.vector.tensor_copy(qt_sb, qtp[:, :dq])
                    self.qt_sb = qt_sb
                    self.acc = accbuf.tile([128, dq], f32, tag="acc")
                    self.ot_ps = ps_ot.tile([128, dq], f32, tag="ot")

                def fold_partial(self, part):
                    if self.pending:
                        prev, depth = self.pending.pop()
                        if depth < 2:
                            comb = pairbuf.tile([128, dq], bf16, tag="pair")
                            nc.gpsimd.tensor_add(comb, prev, part)
                            self.pending.append((comb, depth + 1))
                        else:
                            nc.vector.tensor_add(self.acc, self.acc, prev)
                            self.pending.append((part, 1))
                    else:
                        self.pending.append((part, 1))

                def emit_scores_pair(self, p):
                    i = self.i
                    ts_ = [t for t in (2 * p, 2 * p + 1) if t <= i]
                    w = len(ts_) * dq
                    st = ps_s.tile([128, 2 * dq], f32, tag="st")
                    for s, t in enumerate(ts_):
                        nc.tensor.matmul(
                            st[:, s * dq : s * dq + dq],
                            kt_sbs[self.b][:, t * 128 : (t + 1) * 128],
                            self.qt_sb,
                            start=True,
                            stop=(t != i),
                        )
                        if t == i:
                            nc.tensor.matmul(
                                st[:, s * dq : s * dq + dq],
                                maskt,
                                ident_rep,
                                start=False,
                                stop=True,
                            )
                    pt = pbuf.tile([128, 2 * dq], bf16, tag="pt")
                    nc.scalar.activation(
                        pt[:, :w], st[:, :w], AF.Exp, bias=0.0, scale=SCALING
                    )
                    self.pts[p] = pt
                    acc = self.acc
                    if len(ts_) == 2:
                        if p == 0:
                            nc.vector.tensor_add(
                                acc, pt[:, :dq], pt[:, dq : 2 * dq]
                            )
                        else:
                            tmp = pairbuf.tile([128, dq], bf16, tag="pair")
                            nc.gpsimd.tensor_add(
                                tmp, pt[:, :dq], pt[:, dq : 2 * dq]
                            )
                            self.fold_partial(tmp)
                    else:
                        if p == 0:
                            nc.vector.tensor_copy(acc, pt[:, :dq])
                        else:
                            self.fold_partial(pt[:, :dq])

                def emit_pv_pair(self, p):
                    i = self.i
                    pt = self.pts.pop(p)
                    for s, t in enumerate(
                        tt for tt in (2 * p, 2 * p + 1) if tt <= i
                    ):
                        nc.tensor.matmul(
                            self.ot_ps,
                            v_sbs[self.b][:, t, :],
                            pt[:, s * dq : s * dq + dq],
                            start=(t == 0),
                            stop=(t == i),
                        )

                def finish_acc(self):
                    for part, _ in self.pending:
                        nc.vector.tensor_add(self.acc, self.acc, part)
                    self.pending = []

                def emit_tail(self):
                    i = self.i
                    o_sb = obuf.tile([128, dq], f32, tag="o_sb")
                    sums_ps = ps_sum.tile([128, n_heads], f32, tag="sums")
                    for j in range(n_heads):
                        nc.tensor.matmul(
                            sums_ps[:, j : j + 1],
                            self.acc[:, j * 128 : (j + 1) * 128],
                            ones_f32,
                            start=(j == 0),
                            stop=(j == n_heads - 1),
                        )
                    recip = stats.tile([128, n_heads], f32, tag="recip")
                    nc.vector.reciprocal(recip, sums_ps)

                    ot_sb = otbuf.tile([128, dq], f32, tag="ot_sb")
                    nc.vector.tensor_copy(ot_sb, self.ot_ps)
                    otr = ps_tp.tile([128, dq], f32, tag="tp")
                    transpose_group(
                        otr,
                        [
                            ot_sb[:, j * 128 : (j + 1) * 128]
                            for j in range(n_heads)
                        ],
                        ident_f32,
                    )
                    for j in range(n_heads):
                        nc.vector.tensor_scalar_mul(
                            o_sb[:, j * 128 : (j + 1) * 128],
                            otr[:, j * 128 : (j + 1) * 128],
                            recip[:, j : j + 1],
                        )
                    nc.sync.dma_start(
                        out=o[self.b, i * 128 : (i + 1) * 128, :], in_=o_sb
                    )

            pending_tail = None
            for b in range(batch):
                for i in range(nt):
                    it = Item(b, i)
                    it.emit_scores_pair(0)
                    for p in range(it.npair):
                        if p + 1 < it.npair:
                            it.emit_scores_pair(p + 1)
                        it.emit_pv_pair(p)
                    it.finish_acc()
                    if pending_tail is not None:
                        pending_tail.emit_tail()
                    pending_tail = it
            pending_tail.emit_tail()

        if reps > 1:
            with tc.For_i(0, reps, 1) as _it:
                body(_it)
        else:
            body()

    nc.compile()
    return nc


_PROGRAM = None
SCHEME = "A"


def _get_program():
    global _PROGRAM
    if _PROGRAM is None:
        if SCHEME == "B":
            _PROGRAM = build_program_b()
        else:
            _PROGRAM = build_program()
    return _PROGRAM


def kernel(query, key, value):
    from concourse.bass_utils import run_bass_kernel_spmd

    query = np.asarray(query, dtype=np.float32)
    key = np.asarray(key, dtype=np.float32)
    value = np.asarray(value, dtype=np.float32)
    assert query.shape == (B, S, N_CORES * DQ), query.shape
    assert key.shape == (B, S, N_CORES * D), key.shape
    assert value.shape == (B, S, N_CORES * D), value.shape

    nc = _get_program()
    in_maps = [
        {
            "q": np.ascontiguousarray(query[:, :, c * DQ : (c + 1) * DQ]),
            "k": np.ascontiguousarray(key[:, :, c * D : (c + 1) * D]),
            "v": np.ascontiguousarray(value[:, :, c * D : (c + 1) * D]),
        }
        for c in range(N_CORES)
    ]
    res = run_bass_kernel_spmd(nc, in_maps, core_ids=list(range(N_CORES)))
    out = np.empty((B, S, N_CORES * DQ), dtype=np.float32)
    for c in range(N_CORES):
        out[:, :, c * DQ : (c + 1) * DQ] = res.results[c]["o"]
    return out
